# revision 25
# baseline (speedup 1.0000x reference)
"""Modulated deformable conv (DCNv2) Trainium2 Bass kernel.

Sharding: 8 cores = 4 batches x 2 pixel-halves (image rows 0-63 / 64-127).

Host prep (data-independent): xq[y*128+x] = corner quad
  [x[:,y,x], x[:,y,x+1], x[:,y+1,x], x[:,y+1,x+1]] -> [16384, 512] f16 per
  batch, so ONE gather index fetches all 4 bilinear corners of one tap.

Per core:
  B. Offset/mask convs as 9 shift-matmuls in PSUM -> [27, 8192] f32.
  C. PE-transpose conv out to pixel-partition layout [128pp, 27, 64blk];
     compute per-tap quad weights wt[128, 4q, 9k, 64blk] (f16, stays in
     SBUF) and quad indices idx = clamp(y0,0,126)*128 + clamp(x0,0,126)
     with slot-select weights handling the clamp; stage indices via DRAM
     into dma_gather wrap layout [128, 9k, 4ch, 128].
  D. For each chunk (4 x 2048 px) and tap k: non-transpose dma_gather of
     quads -> gt[128pp, 16slot, 512]; DVE-combine 4 corners with wt ->
     val[128pp, 16, 128c]; PE-transpose -> [128c, 2048px]; scalar-copy to
     SBUF f16; matmul w2 -> accumulate out PSUM [128o, 2048] over 9 taps.
Pixel halves are disjoint; the host just concatenates the 8 outputs.
"""

import numpy as np

import concourse.bass as bass
import concourse.tile as tile
from concourse import bacc, mybir
from concourse.bass_utils import run_bass_kernel_spmd
from concourse.masks import make_identity

f16 = mybir.dt.float16
f32 = mybir.dt.float32
i16 = mybir.dt.int16
i32 = mybir.dt.int32
Alu = mybir.AluOpType
Act = mybir.ActivationFunctionType

H = W = 128
HW = H * W
C = 128
O = 128
K = 9
NCH = 27          # conv output channels: [off_y(9), off_x(9), mask_logit(9)]
NPX = HW // 2     # pixels per core (one half: 64 image rows)
BLK = NPX // 128  # 64 local row-blocks
CHUNK = 2048      # pixels per PSUM pass
NCHUNK = NPX // CHUNK  # 4
SLOTS = CHUNK // 128   # 16 row-blocks per chunk


def _ap(src_ap, offset, pattern):
    """Raw AP at an element offset relative to an existing (DRAM) AP."""
    return bass.AP(tensor=src_ap.tensor, offset=src_ap.offset + offset,
                   ap=[list(p) for p in pattern])


def _apf(src_ap, offset, free_pattern):
    """SBUF/PSUM AP: keep the tile's partition dim, replace free dims."""
    return bass.AP(tensor=src_ap.tensor, offset=src_ap.offset + offset,
                   ap=[list(src_ap.ap[0])] + [list(p) for p in free_pattern])


def build_kernel(debug=False):
    nc = bacc.Bacc("TRN2", target_bir_lowering=False, debug=False,
                   enable_asserts=True, dynamic_dma_scratch_size=32768)

    # ---- I/O ----
    xq_in = nc.dram_tensor("xq", [HW * 4 * C], f16, kind="ExternalInput")
    xpad_in = nc.dram_tensor("xpad", [C, 66 * 130], f16, kind="ExternalInput")
    wconv_in = nc.dram_tensor("wconv", [C, K * NCH], f16, kind="ExternalInput")
    bias_in = nc.dram_tensor("bias", [NCH, 1], f32, kind="ExternalInput")
    w2_in = nc.dram_tensor("w2", [C, K * O], f16, kind="ExternalInput")
    basey_in = nc.dram_tensor("basey", [128, K * BLK], f32, kind="ExternalInput")
    basex_in = nc.dram_tensor("basex", [128, K], f32, kind="ExternalInput")
    out_o = nc.dram_tensor("out", [O, NPX], f16, kind="ExternalOutput")

    idx_d = nc.dram_tensor("idx_d", [128 * K * BLK], i16)  # [pp, k, blk]

    if debug:
        dbg_conv = nc.dram_tensor("dbg_conv", [NCH, NPX], f32, kind="ExternalOutput")
        dbg_wt = nc.dram_tensor("dbg_wt", [128, K * BLK * 4], f16, kind="ExternalOutput")
        dbg_idx = nc.dram_tensor("dbg_idx", [128, K * BLK], i16, kind="ExternalOutput")
        dbg_g = nc.dram_tensor("dbg_g", [128, SLOTS * 512], f16, kind="ExternalOutput")
        dbg_val = nc.dram_tensor("dbg_val", [128, SLOTS * 128], f32, kind="ExternalOutput")

    with tile.TileContext(nc) as tc:
        with tc.tile_pool(name="persist", bufs=1) as persist:
            w2_t = persist.tile([C, K, O], f16)
            nc.sync.dma_start(w2_t[:], w2_in.ap())
            wt_t = persist.tile([128, K, BLK, 4], f16)
            idx_sbs = [persist.tile([128, K, 256], i16, name=f"idx_sb{g}")
                       for g in range(2)]
            identp = persist.tile([128, 128], f32)
            make_identity(nc, identp[:])

            # ========= Phases B+C, pipelined per 16-blk chunk =========
            with tc.tile_pool(name="convph", bufs=1) as cph, \
                 tc.tile_pool(name="psconv", bufs=4, space="PSUM") as psc, \
                 tc.tile_pool(name="wmath", bufs=1) as wm, \
                 tc.tile_pool(name="pst", bufs=2, space="PSUM") as pst:
                xpad_t = cph.tile([C, 66, 130], f16)
                # split the image load so chunk 0's conv starts early
                for r0, r1 in ((0, 18), (18, 34), (34, 50), (50, 66)):
                    nc.sync.dma_start(
                        xpad_t[:, r0:r1, :],
                        _ap(xpad_in.ap(), r0 * 130,
                            [[66 * 130, C], [1, (r1 - r0) * 130]]))
                wconv_t = cph.tile([C, K, NCH], f16)
                nc.sync.dma_start(wconv_t[:], wconv_in.ap())
                bias_t = cph.tile([NCH, 1], f32)
                nc.sync.dma_start(bias_t[:], bias_in.ap())
                conv_sb = cph.tile([NCH, NPX], f32)
                ident = cph.tile([128, 128], f32)
                make_identity(nc, ident[:])
                basey_t = wm.tile([128, K, BLK], f32)
                nc.sync.dma_start(basey_t[:], basey_in.ap())
                basex_t = wm.tile([128, K], f32)
                nc.sync.dma_start(basex_t[:], basex_in.ap())

                NG = 2              # staging groups
                GB = BLK // NG      # 32 blocks per group
                shp = [128, K, GB]

                def scratch(tag):
                    return wm.tile(shp, f32, tag=tag, name="sc_" + tag)

                for g in range(NG):
                    # ---- conv for this group's 32 image rows ----
                    for t in range(8 * g, 8 * g + 8):  # 512 px (4 rows) each
                        ps = psc.tile([NCH, 512], f32)
                        for k in range(K):
                            ki, kj = k // 3, k % 3
                            rhs = _apf(xpad_t[:], (t * 4 + ki) * 130 + kj,
                                       [[130, 4], [1, 128]])
                            nc.tensor.matmul(ps[:], wconv_t[:, k, :], rhs,
                                             start=(k == 0), stop=(k == K - 1))
                        nc.scalar.activation(conv_sb[:, t * 512:(t + 1) * 512],
                                             ps[:], Act.Identity,
                                             bias=bias_t[:, 0:1])

                    # ---- transpose to pixel-partition ----
                    offs = wm.tile([128, NCH, GB], f32, tag="offs", name="offs")
                    for half in range(2):
                        ps = pst.tile([128, 16 * NCH], f32)
                        for j in range(16):
                            blk = g * GB + half * 16 + j
                            nc.tensor.transpose(
                                ps[:, j * NCH:(j + 1) * NCH],
                                conv_sb[:, blk * 128:(blk + 1) * 128],
                                ident[0:NCH, 0:NCH])
                        nc.scalar.activation(
                            _apf(offs[:], half * 16, [[GB, NCH], [1, 16]]),
                            _apf(ps[:], 0, [[1, NCH], [NCH, 16]]), Act.Copy)

                    # ---- weight/index math on [128, K, 32] ----
                    # validity of out-of-image corners is encoded by the
                    # clamp-delta indicators below, so no explicit masks.
                    off_y = offs[:, 0:K, :]
                    off_x = offs[:, K:2 * K, :]
                    logits = offs[:, 2 * K:3 * K, :]

                    py = scratch("py")
                    nc.vector.tensor_tensor(py[:], off_y,
                                            basey_t[:, :, g * GB:(g + 1) * GB],
                                            Alu.add)
                    px = scratch("px")
                    bx_b = basex_t[:, :, None].to_broadcast(tuple(shp))
                    nc.vector.tensor_tensor(px[:], off_x, bx_b, Alu.add)

                    def floor_(v, tag):
                        ri = wm.tile(shp, i32, tag="ri", name="ri")
                        nc.vector.tensor_copy(ri[:], v[:])
                        rf = scratch("rf")
                        nc.vector.tensor_copy(rf[:], ri[:])
                        gt = scratch("gt")
                        nc.vector.tensor_tensor(gt[:], rf[:], v[:], Alu.is_gt)
                        out = scratch(tag)
                        nc.vector.tensor_tensor(out[:], rf[:], gt[:], Alu.subtract)
                        return out

                    y0 = floor_(py, "y0")
                    x0 = floor_(px, "x0")
                    wy1 = scratch("wy1")
                    nc.vector.tensor_tensor(wy1[:], py[:], y0[:], Alu.subtract)
                    wx1 = scratch("wx1")
                    nc.vector.tensor_tensor(wx1[:], px[:], x0[:], Alu.subtract)
                    wy0 = scratch("wy0")
                    nc.vector.tensor_scalar(wy0[:], wy1[:], -1.0, 1.0, Alu.mult, Alu.add)
                    wx0 = scratch("wx0")
                    nc.vector.tensor_scalar(wx0[:], wx1[:], -1.0, 1.0, Alu.mult, Alu.add)

                    msig = scratch("msig")
                    nc.scalar.activation(msig[:], logits, Act.Sigmoid)
                    A0 = scratch("A0")
                    nc.vector.tensor_tensor(A0[:], wy0[:], msig[:], Alu.mult)
                    A1 = scratch("A1")
                    nc.vector.tensor_tensor(A1[:], wy1[:], msig[:], Alu.mult)

                    # slot-select weights for a clamped base b = clamp(v0,0,126):
                    # slot0 covers row b (corner v0 iff d==0, corner v0+1 iff
                    # d==-1), slot1 covers row b+1 (corner v0+1 iff d==0,
                    # corner v0 iff d==1); d outside {-1,0,1} zeroes both,
                    # which also drops every out-of-image corner.
                    def slot_weights(v0, W0, W1, tag):
                        b = scratch("b" + tag)
                        nc.vector.tensor_scalar(b[:], v0[:], 0.0, 126.0, Alu.max, Alu.min)
                        d = scratch("d" + tag)
                        nc.vector.tensor_tensor(d[:], v0[:], b[:], Alu.subtract)
                        e0 = scratch("e0" + tag)
                        nc.vector.tensor_scalar(e0[:], d[:], 0.0, None, Alu.is_equal)
                        em = scratch("em" + tag)
                        nc.vector.tensor_scalar(em[:], d[:], -1.0, None, Alu.is_equal)
                        ep = scratch("ep" + tag)
                        nc.vector.tensor_scalar(ep[:], d[:], 1.0, None, Alu.is_equal)
                        ws0 = scratch("ws0" + tag)
                        t1 = scratch("t1" + tag)
                        nc.vector.tensor_tensor(ws0[:], W0[:], e0[:], Alu.mult)
                        nc.vector.tensor_tensor(t1[:], W1[:], em[:], Alu.mult)
                        nc.vector.tensor_tensor(ws0[:], ws0[:], t1[:], Alu.add)
                        ws1 = scratch("ws1" + tag)
                        t2 = scratch("t2" + tag)
                        nc.vector.tensor_tensor(ws1[:], W1[:], e0[:], Alu.mult)
                        nc.vector.tensor_tensor(t2[:], W0[:], ep[:], Alu.mult)
                        nc.vector.tensor_tensor(ws1[:], ws1[:], t2[:], Alu.add)
                        return b, ws0, ws1

                    by, wsy0, wsy1 = slot_weights(y0, A0, A1, "y")
                    bx, wsx0, wsx1 = slot_weights(x0, wx0, wx1, "x")

                    # quad weights wt[..., q=2*sy+sx] = wsy_sy * wsx_sx  (f16,
                    # q innermost for the Phase D combine)
                    for sy, Wy in ((0, wsy0), (1, wsy1)):
                        for sx, Wx in ((0, wsx0), (1, wsx1)):
                            nc.vector.tensor_tensor(
                                wt_t[:, :, g * GB:(g + 1) * GB, sy * 2 + sx],
                                Wy[:], Wx[:], Alu.mult)

                    # quad index = by*128 + bx
                    idxf = scratch("idxf")
                    nc.vector.tensor_scalar(idxf[:], by[:], 128.0, None, Alu.mult)
                    nc.vector.tensor_tensor(idxf[:], idxf[:], bx[:], Alu.add)
                    idx16 = wm.tile(shp, i16, tag="idx16", name="idx16")
                    nc.vector.tensor_copy(idx16[:], idxf[:])

                    # stage to DRAM [pp, k, blk-slice], reload [16p, k, h,
                    # blk] (blk innermost keeps runs contiguous), DVE-permute
                    # to gather order j = slot*128 + h*16 + p = slot*128 + pp,
                    # replicate x8 into the 128-partition wrap layout.
                    nc.sync.dma_start(
                        _ap(idx_d.ap(), g * GB,
                            [[K * BLK, 128], [BLK, K], [1, GB]]),
                        idx16[:])
                    idx_raw = wm.tile([16, K, 8, GB], i16, tag="iraw",
                                      name="idx_raw")
                    nc.sync.dma_start(
                        idx_raw[:],
                        _ap(idx_d.ap(), g * GB,
                            [[K * BLK, 16], [BLK, K], [16 * K * BLK, 8],
                             [1, GB]]))
                    idx_tr = wm.tile([16, K, 2, 16, 8], i16, tag="itr",
                                     name="idx_tr")
                    for k in range(K):
                        src = bass.AP(tensor=idx_raw[:].tensor,
                                      offset=idx_raw[:].offset + k * 8 * GB,
                                      ap=[list(idx_raw[:].ap[0]),
                                          [16, 2], [1, 16], [GB, 8]])
                        nc.vector.tensor_copy(idx_tr[:, k, :, :, :], src)
                    for g8 in range(8):
                        nc.sync.dma_start(
                            idx_sbs[g][g8 * 16:(g8 + 1) * 16, :, :],
                            idx_tr[:].rearrange("p k c s h -> p k (c s h)"))

                if debug:
                    nc.sync.dma_start(dbg_conv.ap(), conv_sb[:])
                    nc.sync.dma_start(dbg_wt.ap(), wt_t[:])
                    nc.sync.dma_start(
                        _ap(dbg_idx.ap(), 0, [[1, 128 * K * BLK]]),
                        _ap(idx_d.ap(), 0, [[1, 128 * K * BLK]]))

            # ============ Phase D: gather + combine + GEMM ============
            with tc.tile_pool(name="gath", bufs=4) as gp, \
                 tc.tile_pool(name="vp", bufs=2) as vp, \
                 tc.tile_pool(name="vtp", bufs=2) as vtp, \
                 tc.tile_pool(name="oev", bufs=2) as op_, \
                 tc.tile_pool(name="pstr", bufs=1, space="PSUM") as pstr, \
                 tc.tile_pool(name="psout", bufs=1, space="PSUM") as pso:
                for ch in range(NCHUNK):
                    out_ps = pso.tile([O, CHUNK], f32)
                    for k in range(K):
                        gt = gp.tile([128, SLOTS, 512], f16, tag="g")
                        in_ap = _ap(xq_in.ap(), 0, [[512, HW], [1, 512]])
                        out_ap = _apf(gt[:], 0, [[512, SLOTS], [1, 512]])
                        idxs = idx_sbs[ch // 2][:, k,
                                                (ch % 2) * 128:(ch % 2 + 1) * 128]
                        nc.gpsimd.dma_gather(out_ap, in_ap, idxs,
                                             num_idxs=CHUNK, num_idxs_reg=CHUNK,
                                             elem_size=512, elem_step=512,
                                             transpose=False,
                                             single_packet=False)
                        if debug and ch == 0 and k == 0:
                            nc.sync.dma_start(dbg_g.ap(), gt[:])
                        # gt element layout is [c, q] (q innermost): one 2x-mode
                        # multiply by the broadcast quad weights, then an
                        # innermost-axis add-reduce over q.
                        tmp = vp.tile([128, SLOTS, 128, 4], f16, tag="t")
                        val = vp.tile([128, SLOTS, 128], f32, tag="v")
                        wb = wt_t[:, k, ch * SLOTS:(ch + 1) * SLOTS, :][
                            :, :, None, :].to_broadcast((128, SLOTS, 128, 4))
                        gq = _apf(gt[:], 0, [[512, SLOTS], [4, 128], [1, 4]])
                        nc.vector.tensor_tensor(tmp[:], gq, wb, Alu.mult)
                        nc.vector.tensor_reduce(val[:], tmp[:],
                                                mybir.AxisListType.X, Alu.add)
                        if debug and ch == 0 and k == 0:
                            nc.sync.dma_start(dbg_val.ap(), val[:])
                        psT = pstr.tile([128, CHUNK], f32)
                        for j in range(SLOTS):
                            nc.tensor.transpose(psT[:, j * 128:(j + 1) * 128],
                                                val[:, j, :], identp[:])
                        valT = vtp.tile([128, CHUNK], f16, tag="vt")
                        nc.scalar.activation(valT[:], psT[:], Act.Copy)
                        for b in range(CHUNK // 512):
                            nc.tensor.matmul(
                                out_ps[:, b * 512:(b + 1) * 512],
                                w2_t[:, k, :],
                                valT[:, b * 512:(b + 1) * 512],
                                start=(k == 0), stop=(k == K - 1))
                    ot = op_.tile([O, CHUNK], f16, tag="o")
                    nc.scalar.activation(ot[:], out_ps[:], Act.Copy)
                    nc.sync.dma_start(
                        _ap(out_o.ap(), ch * CHUNK, [[NPX, O], [1, CHUNK]]),
                        ot[:])
    nc.compile()
    return nc


def _host_inputs(x, w_off, b_off, w_mod, b_mod, w_reg):
    """Build the 8 per-core input maps."""
    # conv weights reordered: [off_y(9), off_x(9), mask(9)]
    wcat = np.concatenate([w_off[0::2], w_off[1::2], w_mod], axis=0)  # [27,128,3,3]
    bcat = np.concatenate([b_off[0::2], b_off[1::2], b_mod], axis=0)  # [27]
    wconv = np.ascontiguousarray(
        wcat.transpose(1, 2, 3, 0).reshape(C, K * NCH)).astype(np.float16)
    bias = bcat.reshape(NCH, 1).astype(np.float32)
    w2 = np.ascontiguousarray(
        (w_reg * 2.0).transpose(1, 2, 3, 0).reshape(C, K * O)).astype(np.float16)
    ki = np.arange(K) // 3
    kj = np.arange(K) % 3
    basex = (np.arange(128)[:, None] + kj[None, :] - 1).astype(np.float32)

    # corner-quad layout per batch, q innermost: xq[y*128+x][c][q] with
    # q = [x(y,x), x(y,x+1), x(y+1,x), x(y+1,x+1)][c]
    B = x.shape[0]
    xf = x.astype(np.float16)
    xq_all = []
    for b in range(B):
        xp = np.zeros((129, 129, C), dtype=np.float16)
        xp[:128, :128] = xf[b].transpose(1, 2, 0)
        quad = np.stack([xp[:128, :128], xp[:128, 1:129],
                         xp[1:129, :128], xp[1:129, 1:129]], axis=-1)
        xq_all.append(np.ascontiguousarray(quad.reshape(HW * 4 * C)))

    maps = []
    for core in range(8):
        b, hf = core // 2, core % 2
        xpadfull = np.zeros((C, 130, 130), dtype=np.float16)
        xpadfull[:, 1:129, 1:129] = xf[b]
        xpad = np.ascontiguousarray(xpadfull[:, 64 * hf:64 * hf + 66, :])
        rloc = 64 * hf + np.arange(BLK)
        basey = np.broadcast_to(
            (rloc[None, :] + ki[:, None] - 1)[None, :, :],
            (128, K, BLK)).reshape(128, K * BLK).astype(np.float32)
        maps.append({
            "xq": xq_all[b],
            "xpad": xpad.reshape(C, 66 * 130),
            "wconv": wconv,
            "bias": bias,
            "w2": w2,
            "basey": np.ascontiguousarray(basey),
            "basex": basex,
        })
    return maps


_NC_CACHE = {}


def kernel(x, w_off, b_off, w_mod, b_mod, w_reg, debug=False, trace=False):
    x = np.asarray(x)
    key = ("nc", debug)
    if key not in _NC_CACHE:
        _NC_CACHE[key] = build_kernel(debug=debug)
    nc = _NC_CACHE[key]
    maps = _host_inputs(x, np.asarray(w_off), np.asarray(b_off),
                        np.asarray(w_mod), np.asarray(b_mod), np.asarray(w_reg))
    res = run_bass_kernel_spmd(nc, maps, core_ids=list(range(8)), trace=trace)
    B = x.shape[0]
    out = np.empty((B, O, H, W), dtype=np.float32)
    for core in range(8):
        b, hf = core // 2, core % 2
        out[b, :, 64 * hf:64 * (hf + 1), :] = \
            res.results[core]["out"].astype(np.float32).reshape(O, BLK, 128)
    kernel._last_results = res
    return out


# revision 26
# speedup vs baseline: 1.1766x; 1.1766x over previous
"""Modulated deformable conv (DCNv2) Trainium2 Bass kernel.

Sharding: 8 cores = 4 batches x 2 pixel-halves (image rows 0-63 / 64-127).

Host prep (data-independent): xq[y*128+x] = corner quad
  [x[:,y,x], x[:,y,x+1], x[:,y+1,x], x[:,y+1,x+1]] -> [16384, 512] f16 per
  batch, so ONE gather index fetches all 4 bilinear corners of one tap.

Per core:
  B. Offset/mask convs as 9 shift-matmuls in PSUM -> [27, 8192] f32.
  C. PE-transpose conv out to pixel-partition layout [128pp, 27, 64blk];
     compute per-tap quad weights wt[128, 4q, 9k, 64blk] (f16, stays in
     SBUF) and quad indices idx = clamp(y0,0,126)*128 + clamp(x0,0,126)
     with slot-select weights handling the clamp; stage indices via DRAM
     into dma_gather wrap layout [128, 9k, 4ch, 128].
  D. For each chunk (4 x 2048 px) and tap k: non-transpose dma_gather of
     quads -> gt[128pp, 16slot, 512]; DVE-combine 4 corners with wt ->
     val[128pp, 16, 128c]; PE-transpose -> [128c, 2048px]; scalar-copy to
     SBUF f16; matmul w2 -> accumulate out PSUM [128o, 2048] over 9 taps.
Pixel halves are disjoint; the host just concatenates the 8 outputs.
"""

import numpy as np

import concourse.bass as bass
import concourse.tile as tile
from concourse import bacc, mybir
from concourse.bass_utils import run_bass_kernel_spmd
from concourse.masks import make_identity

f16 = mybir.dt.float16
f32 = mybir.dt.float32
i16 = mybir.dt.int16
i32 = mybir.dt.int32
Alu = mybir.AluOpType
Act = mybir.ActivationFunctionType

H = W = 128
HW = H * W
C = 128
O = 128
K = 9
NCH = 27          # conv output channels: [off_y(9), off_x(9), mask_logit(9)]
NPX = HW // 2     # pixels per core (one half: 64 image rows)
BLK = NPX // 128  # 64 local row-blocks
CHUNK = 2048      # pixels per PSUM pass
NCHUNK = NPX // CHUNK  # 4
SLOTS = CHUNK // 128   # 16 row-blocks per chunk


def _ap(src_ap, offset, pattern):
    """Raw AP at an element offset relative to an existing (DRAM) AP."""
    return bass.AP(tensor=src_ap.tensor, offset=src_ap.offset + offset,
                   ap=[list(p) for p in pattern])


def _apf(src_ap, offset, free_pattern):
    """SBUF/PSUM AP: keep the tile's partition dim, replace free dims."""
    return bass.AP(tensor=src_ap.tensor, offset=src_ap.offset + offset,
                   ap=[list(src_ap.ap[0])] + [list(p) for p in free_pattern])


def build_kernel(debug=False):
    nc = bacc.Bacc("TRN2", target_bir_lowering=False, debug=False,
                   enable_asserts=True, dynamic_dma_scratch_size=32768)

    # ---- I/O ----
    xq_in = nc.dram_tensor("xq", [HW * 4 * C], f16, kind="ExternalInput")
    xpad_in = nc.dram_tensor("xpad", [C, 66 * 130], f16, kind="ExternalInput")
    wconv_in = nc.dram_tensor("wconv", [C, K * NCH], f16, kind="ExternalInput")
    bias_in = nc.dram_tensor("bias", [NCH, 1], f32, kind="ExternalInput")
    w2_in = nc.dram_tensor("w2", [C, K * O], f16, kind="ExternalInput")
    basey_in = nc.dram_tensor("basey", [128, K * BLK], f32, kind="ExternalInput")
    basex_in = nc.dram_tensor("basex", [128, K], f32, kind="ExternalInput")
    out_o = nc.dram_tensor("out", [O, NPX], f16, kind="ExternalOutput")

    idx_d = nc.dram_tensor("idx_d", [128 * K * BLK], i16)  # [pp, k, blk]

    if debug:
        dbg_conv = nc.dram_tensor("dbg_conv", [NCH, NPX], f32, kind="ExternalOutput")
        dbg_wt = nc.dram_tensor("dbg_wt", [128, K * BLK * 4], f16, kind="ExternalOutput")
        dbg_idx = nc.dram_tensor("dbg_idx", [128, K * BLK], i16, kind="ExternalOutput")
        dbg_g = nc.dram_tensor("dbg_g", [128, SLOTS * 512], f16, kind="ExternalOutput")
        dbg_val = nc.dram_tensor("dbg_val", [128, SLOTS * 128], f32, kind="ExternalOutput")

    with tile.TileContext(nc) as tc:
        with tc.tile_pool(name="persist", bufs=1) as persist:
            w2_t = persist.tile([C, K, O], f16)
            nc.sync.dma_start(w2_t[:], w2_in.ap())
            wt_t = persist.tile([128, K, BLK, 4], f16)
            idx_sb = persist.tile([128, K, NCHUNK * 128], i16)
            identp = persist.tile([128, 128], f32)
            make_identity(nc, identp[:])

            # ========= Phases B+C, pipelined per 16-blk chunk =========
            with tc.tile_pool(name="convph", bufs=1) as cph, \
                 tc.tile_pool(name="psconv", bufs=4, space="PSUM") as psc, \
                 tc.tile_pool(name="wmath", bufs=1) as wm, \
                 tc.tile_pool(name="pst", bufs=2, space="PSUM") as pst:
                xpad_t = cph.tile([C, 66, 130], f16)
                # split the image load so chunk 0's conv starts early
                for r0, r1 in ((0, 18), (18, 34), (34, 50), (50, 66)):
                    nc.sync.dma_start(
                        xpad_t[:, r0:r1, :],
                        _ap(xpad_in.ap(), r0 * 130,
                            [[66 * 130, C], [1, (r1 - r0) * 130]]))
                wconv_t = cph.tile([C, K, NCH], f16)
                nc.sync.dma_start(wconv_t[:], wconv_in.ap())
                bias_t = cph.tile([NCH, 1], f32)
                nc.sync.dma_start(bias_t[:], bias_in.ap())
                conv_sb = cph.tile([NCH, NPX], f32)
                ident = cph.tile([128, 128], f32)
                make_identity(nc, ident[:])
                basey_t = wm.tile([128, K, BLK], f32)
                nc.sync.dma_start(basey_t[:], basey_in.ap())
                basex_t = wm.tile([128, K], f32)
                nc.sync.dma_start(basex_t[:], basex_in.ap())

                NG = 2              # staging groups
                GB = BLK // NG      # 32 blocks per group
                shp = [128, K, GB]

                def scratch(tag):
                    return wm.tile(shp, f32, tag=tag, name="sc_" + tag)

                for g in range(NG):
                    # ---- conv for this group's 32 image rows ----
                    for t in range(8 * g, 8 * g + 8):  # 512 px (4 rows) each
                        ps = psc.tile([NCH, 512], f32)
                        for k in range(K):
                            ki, kj = k // 3, k % 3
                            rhs = _apf(xpad_t[:], (t * 4 + ki) * 130 + kj,
                                       [[130, 4], [1, 128]])
                            nc.tensor.matmul(ps[:], wconv_t[:, k, :], rhs,
                                             start=(k == 0), stop=(k == K - 1))
                        nc.scalar.activation(conv_sb[:, t * 512:(t + 1) * 512],
                                             ps[:], Act.Identity,
                                             bias=bias_t[:, 0:1])

                    # ---- transpose to pixel-partition ----
                    offs = wm.tile([128, NCH, GB], f32, tag="offs", name="offs")
                    for half in range(2):
                        ps = pst.tile([128, 16 * NCH], f32)
                        for j in range(16):
                            blk = g * GB + half * 16 + j
                            nc.tensor.transpose(
                                ps[:, j * NCH:(j + 1) * NCH],
                                conv_sb[:, blk * 128:(blk + 1) * 128],
                                ident[0:NCH, 0:NCH])
                        nc.scalar.activation(
                            _apf(offs[:], half * 16, [[GB, NCH], [1, 16]]),
                            _apf(ps[:], 0, [[1, NCH], [NCH, 16]]), Act.Copy)

                    # ---- weight/index math on [128, K, 32] ----
                    # validity of out-of-image corners is encoded by the
                    # clamp-delta indicators below, so no explicit masks.
                    off_y = offs[:, 0:K, :]
                    off_x = offs[:, K:2 * K, :]
                    logits = offs[:, 2 * K:3 * K, :]

                    py = scratch("py")
                    nc.vector.tensor_tensor(py[:], off_y,
                                            basey_t[:, :, g * GB:(g + 1) * GB],
                                            Alu.add)
                    px = scratch("px")
                    bx_b = basex_t[:, :, None].to_broadcast(tuple(shp))
                    nc.vector.tensor_tensor(px[:], off_x, bx_b, Alu.add)

                    def floor_(v, tag):
                        ri = wm.tile(shp, i32, tag="ri", name="ri")
                        nc.vector.tensor_copy(ri[:], v[:])
                        rf = scratch("rf")
                        nc.vector.tensor_copy(rf[:], ri[:])
                        gt = scratch("gt")
                        nc.vector.tensor_tensor(gt[:], rf[:], v[:], Alu.is_gt)
                        out = scratch(tag)
                        nc.vector.tensor_tensor(out[:], rf[:], gt[:], Alu.subtract)
                        return out

                    y0 = floor_(py, "y0")
                    x0 = floor_(px, "x0")
                    wy1 = scratch("wy1")
                    nc.vector.tensor_tensor(wy1[:], py[:], y0[:], Alu.subtract)
                    wx1 = scratch("wx1")
                    nc.vector.tensor_tensor(wx1[:], px[:], x0[:], Alu.subtract)
                    wy0 = scratch("wy0")
                    nc.vector.tensor_scalar(wy0[:], wy1[:], -1.0, 1.0, Alu.mult, Alu.add)
                    wx0 = scratch("wx0")
                    nc.vector.tensor_scalar(wx0[:], wx1[:], -1.0, 1.0, Alu.mult, Alu.add)

                    msig = scratch("msig")
                    nc.scalar.activation(msig[:], logits, Act.Sigmoid)
                    A0 = scratch("A0")
                    nc.vector.tensor_tensor(A0[:], wy0[:], msig[:], Alu.mult)
                    A1 = scratch("A1")
                    nc.vector.tensor_tensor(A1[:], wy1[:], msig[:], Alu.mult)

                    # slot-select weights for a clamped base b = clamp(v0,0,126):
                    # slot0 covers row b (corner v0 iff d==0, corner v0+1 iff
                    # d==-1), slot1 covers row b+1 (corner v0+1 iff d==0,
                    # corner v0 iff d==1); d outside {-1,0,1} zeroes both,
                    # which also drops every out-of-image corner.
                    def slot_weights(v0, W0, W1, tag):
                        b = scratch("b" + tag)
                        nc.vector.tensor_scalar(b[:], v0[:], 0.0, 126.0, Alu.max, Alu.min)
                        d = scratch("d" + tag)
                        nc.vector.tensor_tensor(d[:], v0[:], b[:], Alu.subtract)
                        e0 = scratch("e0" + tag)
                        nc.vector.tensor_scalar(e0[:], d[:], 0.0, None, Alu.is_equal)
                        em = scratch("em" + tag)
                        nc.vector.tensor_scalar(em[:], d[:], -1.0, None, Alu.is_equal)
                        ep = scratch("ep" + tag)
                        nc.vector.tensor_scalar(ep[:], d[:], 1.0, None, Alu.is_equal)
                        ws0 = scratch("ws0" + tag)
                        t1 = scratch("t1" + tag)
                        nc.vector.tensor_tensor(ws0[:], W0[:], e0[:], Alu.mult)
                        nc.vector.tensor_tensor(t1[:], W1[:], em[:], Alu.mult)
                        nc.vector.tensor_tensor(ws0[:], ws0[:], t1[:], Alu.add)
                        ws1 = scratch("ws1" + tag)
                        t2 = scratch("t2" + tag)
                        nc.vector.tensor_tensor(ws1[:], W1[:], e0[:], Alu.mult)
                        nc.vector.tensor_tensor(t2[:], W0[:], ep[:], Alu.mult)
                        nc.vector.tensor_tensor(ws1[:], ws1[:], t2[:], Alu.add)
                        return b, ws0, ws1

                    by, wsy0, wsy1 = slot_weights(y0, A0, A1, "y")
                    bx, wsx0, wsx1 = slot_weights(x0, wx0, wx1, "x")

                    # quad weights wt[..., q=2*sy+sx] = wsy_sy * wsx_sx  (f16,
                    # q innermost for the Phase D combine)
                    for sy, Wy in ((0, wsy0), (1, wsy1)):
                        for sx, Wx in ((0, wsx0), (1, wsx1)):
                            nc.vector.tensor_tensor(
                                wt_t[:, :, g * GB:(g + 1) * GB, sy * 2 + sx],
                                Wy[:], Wx[:], Alu.mult)

                    # quad index = by*128 + bx
                    idxf = scratch("idxf")
                    nc.vector.tensor_scalar(idxf[:], by[:], 128.0, None, Alu.mult)
                    nc.vector.tensor_tensor(idxf[:], idxf[:], bx[:], Alu.add)
                    idx16 = wm.tile(shp, i16, tag="idx16", name="idx16")
                    nc.vector.tensor_copy(idx16[:], idxf[:])

                    # stage to DRAM [pp, k, blk-slice], reload [16p, k, h,
                    # blk] (blk innermost keeps runs contiguous), DVE-permute
                    # to gather order j = slot*128 + h*16 + p = slot*128 + pp,
                    # replicate x8 into the 128-partition wrap layout.
                    nc.sync.dma_start(
                        _ap(idx_d.ap(), g * GB,
                            [[K * BLK, 128], [BLK, K], [1, GB]]),
                        idx16[:])
                    idx_raw = wm.tile([16, K, 8, GB], i16, tag="iraw",
                                      name="idx_raw")
                    nc.sync.dma_start(
                        idx_raw[:],
                        _ap(idx_d.ap(), g * GB,
                            [[K * BLK, 16], [BLK, K], [16 * K * BLK, 8],
                             [1, GB]]))
                    idx_tr = wm.tile([16, K, 2, 16, 8], i16, tag="itr",
                                     name="idx_tr")
                    for k in range(K):
                        src = bass.AP(tensor=idx_raw[:].tensor,
                                      offset=idx_raw[:].offset + k * 8 * GB,
                                      ap=[list(idx_raw[:].ap[0]),
                                          [16, 2], [1, 16], [GB, 8]])
                        nc.vector.tensor_copy(idx_tr[:, k, :, :, :], src)
                    for g8 in range(8):
                        nc.sync.dma_start(
                            idx_sb[g8 * 16:(g8 + 1) * 16, :,
                                   g * 256:(g + 1) * 256],
                            idx_tr[:].rearrange("p k c s h -> p k (c s h)"))

                if debug:
                    nc.sync.dma_start(dbg_conv.ap(), conv_sb[:])
                    nc.sync.dma_start(dbg_wt.ap(), wt_t[:])
                    nc.sync.dma_start(
                        _ap(dbg_idx.ap(), 0, [[1, 128 * K * BLK]]),
                        _ap(idx_d.ap(), 0, [[1, 128 * K * BLK]]))

            # ============ Phase D: gather + combine + GEMM ============
            with tc.tile_pool(name="gath", bufs=4) as gp, \
                 tc.tile_pool(name="vp", bufs=2) as vp, \
                 tc.tile_pool(name="vtp", bufs=2) as vtp, \
                 tc.tile_pool(name="oev", bufs=2) as op_, \
                 tc.tile_pool(name="pstr", bufs=1, space="PSUM") as pstr, \
                 tc.tile_pool(name="psout", bufs=1, space="PSUM") as pso:
                for ch in range(NCHUNK):
                    out_ps = pso.tile([O, CHUNK], f32)
                    for k in range(K):
                        gt = gp.tile([128, SLOTS, 512], f16, tag="g")
                        in_ap = _ap(xq_in.ap(), 0, [[512, HW], [1, 512]])
                        out_ap = _apf(gt[:], 0, [[512, SLOTS], [1, 512]])
                        nc.gpsimd.dma_gather(out_ap, in_ap,
                                             idx_sb[:, k, ch * 128:(ch + 1) * 128],
                                             num_idxs=CHUNK, num_idxs_reg=CHUNK,
                                             elem_size=512, elem_step=512,
                                             transpose=False,
                                             single_packet=False)
                        if debug and ch == 0 and k == 0:
                            nc.sync.dma_start(dbg_g.ap(), gt[:])
                        # gt element layout is [c, q] (q innermost): one 2x-mode
                        # multiply by the broadcast quad weights, then an
                        # innermost-axis add-reduce over q.
                        tmp = vp.tile([128, SLOTS, 128, 4], f16, tag="t")
                        val = vp.tile([128, SLOTS, 128], f32, tag="v")
                        wb = wt_t[:, k, ch * SLOTS:(ch + 1) * SLOTS, :][
                            :, :, None, :].to_broadcast((128, SLOTS, 128, 4))
                        gq = _apf(gt[:], 0, [[512, SLOTS], [4, 128], [1, 4]])
                        nc.vector.tensor_tensor(tmp[:], gq, wb, Alu.mult)
                        nc.vector.tensor_reduce(val[:], tmp[:],
                                                mybir.AxisListType.X, Alu.add)
                        if debug and ch == 0 and k == 0:
                            nc.sync.dma_start(dbg_val.ap(), val[:])
                        psT = pstr.tile([128, CHUNK], f32)
                        for j in range(SLOTS):
                            nc.tensor.transpose(psT[:, j * 128:(j + 1) * 128],
                                                val[:, j, :], identp[:])
                        valT = vtp.tile([128, CHUNK], f16, tag="vt")
                        nc.scalar.activation(valT[:], psT[:], Act.Copy)
                        for b in range(CHUNK // 512):
                            nc.tensor.matmul(
                                out_ps[:, b * 512:(b + 1) * 512],
                                w2_t[:, k, :],
                                valT[:, b * 512:(b + 1) * 512],
                                start=(k == 0), stop=(k == K - 1))
                    ot = op_.tile([O, CHUNK], f16, tag="o")
                    nc.scalar.activation(ot[:], out_ps[:], Act.Copy)
                    nc.sync.dma_start(
                        _ap(out_o.ap(), ch * CHUNK, [[NPX, O], [1, CHUNK]]),
                        ot[:])
    nc.compile()
    return nc


def _host_inputs(x, w_off, b_off, w_mod, b_mod, w_reg):
    """Build the 8 per-core input maps."""
    # conv weights reordered: [off_y(9), off_x(9), mask(9)]
    wcat = np.concatenate([w_off[0::2], w_off[1::2], w_mod], axis=0)  # [27,128,3,3]
    bcat = np.concatenate([b_off[0::2], b_off[1::2], b_mod], axis=0)  # [27]
    wconv = np.ascontiguousarray(
        wcat.transpose(1, 2, 3, 0).reshape(C, K * NCH)).astype(np.float16)
    bias = bcat.reshape(NCH, 1).astype(np.float32)
    w2 = np.ascontiguousarray(
        (w_reg * 2.0).transpose(1, 2, 3, 0).reshape(C, K * O)).astype(np.float16)
    ki = np.arange(K) // 3
    kj = np.arange(K) % 3
    basex = (np.arange(128)[:, None] + kj[None, :] - 1).astype(np.float32)

    # corner-quad layout per batch, q innermost: xq[y*128+x][c][q] with
    # q = [x(y,x), x(y,x+1), x(y+1,x), x(y+1,x+1)][c]
    B = x.shape[0]
    xf = x.astype(np.float16)
    xq_all = []
    for b in range(B):
        xp = np.zeros((129, 129, C), dtype=np.float16)
        xp[:128, :128] = xf[b].transpose(1, 2, 0)
        quad = np.stack([xp[:128, :128], xp[:128, 1:129],
                         xp[1:129, :128], xp[1:129, 1:129]], axis=-1)
        xq_all.append(np.ascontiguousarray(quad.reshape(HW * 4 * C)))

    maps = []
    for core in range(8):
        b, hf = core // 2, core % 2
        xpadfull = np.zeros((C, 130, 130), dtype=np.float16)
        xpadfull[:, 1:129, 1:129] = xf[b]
        xpad = np.ascontiguousarray(xpadfull[:, 64 * hf:64 * hf + 66, :])
        rloc = 64 * hf + np.arange(BLK)
        basey = np.broadcast_to(
            (rloc[None, :] + ki[:, None] - 1)[None, :, :],
            (128, K, BLK)).reshape(128, K * BLK).astype(np.float32)
        maps.append({
            "xq": xq_all[b],
            "xpad": xpad.reshape(C, 66 * 130),
            "wconv": wconv,
            "bias": bias,
            "w2": w2,
            "basey": np.ascontiguousarray(basey),
            "basex": basex,
        })
    return maps


_NC_CACHE = {}


def kernel(x, w_off, b_off, w_mod, b_mod, w_reg, debug=False, trace=False):
    x = np.asarray(x)
    key = ("nc", debug)
    if key not in _NC_CACHE:
        _NC_CACHE[key] = build_kernel(debug=debug)
    nc = _NC_CACHE[key]
    maps = _host_inputs(x, np.asarray(w_off), np.asarray(b_off),
                        np.asarray(w_mod), np.asarray(b_mod), np.asarray(w_reg))
    res = run_bass_kernel_spmd(nc, maps, core_ids=list(range(8)), trace=trace)
    B = x.shape[0]
    out = np.empty((B, O, H, W), dtype=np.float32)
    for core in range(8):
        b, hf = core // 2, core % 2
        out[b, :, 64 * hf:64 * (hf + 1), :] = \
            res.results[core]["out"].astype(np.float32).reshape(O, BLK, 128)
    kernel._last_results = res
    return out


# revision 29
# speedup vs baseline: 1.1837x; 1.0060x over previous
"""Modulated deformable conv (DCNv2) Trainium2 Bass kernel.

Sharding: 8 cores = 4 batches x 2 pixel-halves (image rows 0-63 / 64-127).

Host prep (data-independent): xq[y*128+x] = corner quad
  [x[:,y,x], x[:,y,x+1], x[:,y+1,x], x[:,y+1,x+1]] -> [16384, 512] f16 per
  batch, so ONE gather index fetches all 4 bilinear corners of one tap.

Per core:
  B. Offset/mask convs as 9 shift-matmuls in PSUM -> [27, 8192] f32.
  C. PE-transpose conv out to pixel-partition layout [128pp, 27, 64blk];
     compute per-tap quad weights wt[128, 4q, 9k, 64blk] (f16, stays in
     SBUF) and quad indices idx = clamp(y0,0,126)*128 + clamp(x0,0,126)
     with slot-select weights handling the clamp; stage indices via DRAM
     into dma_gather wrap layout [128, 9k, 4ch, 128].
  D. For each chunk (4 x 2048 px) and tap k: non-transpose dma_gather of
     quads -> gt[128pp, 16slot, 512]; DVE-combine 4 corners with wt ->
     val[128pp, 16, 128c]; PE-transpose -> [128c, 2048px]; scalar-copy to
     SBUF f16; matmul w2 -> accumulate out PSUM [128o, 2048] over 9 taps.
Pixel halves are disjoint; the host just concatenates the 8 outputs.
"""

import numpy as np

import concourse.bass as bass
import concourse.tile as tile
from concourse import bacc, mybir
from concourse.bass_utils import run_bass_kernel_spmd
from concourse.masks import make_identity

f16 = mybir.dt.float16
f32 = mybir.dt.float32
i16 = mybir.dt.int16
i32 = mybir.dt.int32
Alu = mybir.AluOpType
Act = mybir.ActivationFunctionType

H = W = 128
HW = H * W
C = 128
O = 128
K = 9
NCH = 27          # conv output channels: [off_y(9), off_x(9), mask_logit(9)]
NPX = HW // 2     # pixels per core (one half: 64 image rows)
BLK = NPX // 128  # 64 local row-blocks
CHUNK = 2048      # pixels per PSUM pass
NCHUNK = NPX // CHUNK  # 4
SLOTS = CHUNK // 128   # 16 row-blocks per chunk


def _ap(src_ap, offset, pattern):
    """Raw AP at an element offset relative to an existing (DRAM) AP."""
    return bass.AP(tensor=src_ap.tensor, offset=src_ap.offset + offset,
                   ap=[list(p) for p in pattern])


def _apf(src_ap, offset, free_pattern):
    """SBUF/PSUM AP: keep the tile's partition dim, replace free dims."""
    return bass.AP(tensor=src_ap.tensor, offset=src_ap.offset + offset,
                   ap=[list(src_ap.ap[0])] + [list(p) for p in free_pattern])


def build_kernel(debug=False):
    nc = bacc.Bacc("TRN2", target_bir_lowering=False, debug=False,
                   enable_asserts=True, dynamic_dma_scratch_size=32768)

    # ---- I/O ----
    xq_in = nc.dram_tensor("xq", [HW * 4 * C], f16, kind="ExternalInput")
    xpad_in = nc.dram_tensor("xpad", [C, 66 * 130], f16, kind="ExternalInput")
    wconv_in = nc.dram_tensor("wconv", [C, K * NCH], f16, kind="ExternalInput")
    bias_in = nc.dram_tensor("bias", [NCH, 1], f32, kind="ExternalInput")
    w2_in = nc.dram_tensor("w2", [C, K * O], f16, kind="ExternalInput")
    basey_in = nc.dram_tensor("basey", [128, K * BLK], f32, kind="ExternalInput")
    basex_in = nc.dram_tensor("basex", [128, K], f32, kind="ExternalInput")
    out_o = nc.dram_tensor("out", [O, NPX], f16, kind="ExternalOutput")

    idx_d = nc.dram_tensor("idx_d", [128 * K * BLK], i16)  # [pp, k, blk]

    if debug:
        dbg_conv = nc.dram_tensor("dbg_conv", [NCH, NPX], f32, kind="ExternalOutput")
        dbg_wt = nc.dram_tensor("dbg_wt", [128, K * BLK * 4], f16, kind="ExternalOutput")
        dbg_idx = nc.dram_tensor("dbg_idx", [128, K * BLK], i16, kind="ExternalOutput")
        dbg_g = nc.dram_tensor("dbg_g", [128, SLOTS * 512], f16, kind="ExternalOutput")
        dbg_val = nc.dram_tensor("dbg_val", [128, SLOTS * 128], f32, kind="ExternalOutput")

    with tile.TileContext(nc) as tc:
        with tc.tile_pool(name="persist", bufs=1) as persist:
            w2_t = persist.tile([C, K, O], f16)
            nc.sync.dma_start(w2_t[:], w2_in.ap())
            wt_t = persist.tile([128, K, BLK, 4], f16)
            idx_sb = persist.tile([128, K, NCHUNK * 128], i16)
            identp = persist.tile([128, 128], f32)
            make_identity(nc, identp[:])
            idx_trs = [persist.tile([16, K, 2, 16, 8], i16, name=f"idx_tr{g}")
                       for g in range(2)]

            # ========= Phases B+C, pipelined per 16-blk chunk =========
            with tc.tile_pool(name="convph", bufs=1) as cph, \
                 tc.tile_pool(name="psconv", bufs=4, space="PSUM") as psc, \
                 tc.tile_pool(name="wmath", bufs=1) as wm, \
                 tc.tile_pool(name="pst", bufs=2, space="PSUM") as pst:
                wconv_t = cph.tile([C, K, NCH], f16)
                nc.sync.dma_start(wconv_t[:], wconv_in.ap())
                bias_t = cph.tile([NCH, 1], f32)
                nc.sync.dma_start(bias_t[:], bias_in.ap())
                basey_t = wm.tile([128, K, BLK], f32)
                nc.sync.dma_start(basey_t[:], basey_in.ap())
                basex_t = wm.tile([128, K], f32)
                nc.sync.dma_start(basex_t[:], basex_in.ap())
                xpad_t = cph.tile([C, 66, 130], f16)
                # split the image load so chunk 0's conv starts early
                for r0, r1 in ((0, 6), (6, 18), (18, 34), (34, 50), (50, 66)):
                    nc.sync.dma_start(
                        xpad_t[:, r0:r1, :],
                        _ap(xpad_in.ap(), r0 * 130,
                            [[66 * 130, C], [1, (r1 - r0) * 130]]))
                conv_sb = cph.tile([NCH, NPX], f32)
                ident = cph.tile([128, 128], f32)
                make_identity(nc, ident[:])

                NG = 2              # staging groups
                GB = BLK // NG      # 32 blocks per group
                shp = [128, K, GB]

                def scratch(tag):
                    return wm.tile(shp, f32, tag=tag, name="sc_" + tag)

                for g in range(NG):
                    # ---- conv for this group's 32 image rows ----
                    for t in range(8 * g, 8 * g + 8):  # 512 px (4 rows) each
                        ps = psc.tile([NCH, 512], f32)
                        for k in range(K):
                            ki, kj = k // 3, k % 3
                            rhs = _apf(xpad_t[:], (t * 4 + ki) * 130 + kj,
                                       [[130, 4], [1, 128]])
                            nc.tensor.matmul(ps[:], wconv_t[:, k, :], rhs,
                                             start=(k == 0), stop=(k == K - 1))
                        nc.scalar.activation(conv_sb[:, t * 512:(t + 1) * 512],
                                             ps[:], Act.Identity,
                                             bias=bias_t[:, 0:1])

                    # ---- transpose to pixel-partition ----
                    offs = wm.tile([128, NCH, GB], f32, tag="offs", name="offs")
                    for half in range(2):
                        ps = pst.tile([128, 16 * NCH], f32)
                        for j in range(16):
                            blk = g * GB + half * 16 + j
                            nc.tensor.transpose(
                                ps[:, j * NCH:(j + 1) * NCH],
                                conv_sb[:, blk * 128:(blk + 1) * 128],
                                ident[0:NCH, 0:NCH])
                        nc.scalar.activation(
                            _apf(offs[:], half * 16, [[GB, NCH], [1, 16]]),
                            _apf(ps[:], 0, [[1, NCH], [NCH, 16]]), Act.Copy)

                    # ---- weight/index math on [128, K, 32] ----
                    # validity of out-of-image corners is encoded by the
                    # clamp-delta indicators below, so no explicit masks.
                    off_y = offs[:, 0:K, :]
                    off_x = offs[:, K:2 * K, :]
                    logits = offs[:, 2 * K:3 * K, :]

                    py = scratch("py")
                    nc.vector.tensor_tensor(py[:], off_y,
                                            basey_t[:, :, g * GB:(g + 1) * GB],
                                            Alu.add)
                    px = scratch("px")
                    bx_b = basex_t[:, :, None].to_broadcast(tuple(shp))
                    nc.vector.tensor_tensor(px[:], off_x, bx_b, Alu.add)

                    def floor_(v, tag):
                        ri = wm.tile(shp, i32, tag="ri", name="ri")
                        nc.vector.tensor_copy(ri[:], v[:])
                        rf = scratch("rf")
                        nc.vector.tensor_copy(rf[:], ri[:])
                        gt = scratch("gt")
                        nc.vector.tensor_tensor(gt[:], rf[:], v[:], Alu.is_gt)
                        out = scratch(tag)
                        nc.vector.tensor_tensor(out[:], rf[:], gt[:], Alu.subtract)
                        return out

                    y0 = floor_(py, "y0")
                    x0 = floor_(px, "x0")
                    wy1 = scratch("wy1")
                    nc.vector.tensor_tensor(wy1[:], py[:], y0[:], Alu.subtract)
                    wx1 = scratch("wx1")
                    nc.vector.tensor_tensor(wx1[:], px[:], x0[:], Alu.subtract)
                    wy0 = scratch("wy0")
                    nc.vector.tensor_scalar(wy0[:], wy1[:], -1.0, 1.0, Alu.mult, Alu.add)
                    wx0 = scratch("wx0")
                    nc.vector.tensor_scalar(wx0[:], wx1[:], -1.0, 1.0, Alu.mult, Alu.add)

                    msig = scratch("msig")
                    nc.scalar.activation(msig[:], logits, Act.Sigmoid)
                    A0 = scratch("A0")
                    nc.vector.tensor_tensor(A0[:], wy0[:], msig[:], Alu.mult)
                    A1 = scratch("A1")
                    nc.vector.tensor_tensor(A1[:], wy1[:], msig[:], Alu.mult)

                    # slot-select weights for a clamped base b = clamp(v0,0,126):
                    # slot0 covers row b (corner v0 iff d==0, corner v0+1 iff
                    # d==-1), slot1 covers row b+1 (corner v0+1 iff d==0,
                    # corner v0 iff d==1); d outside {-1,0,1} zeroes both,
                    # which also drops every out-of-image corner.
                    def slot_weights(v0, W0, W1, tag):
                        b = scratch("b" + tag)
                        nc.vector.tensor_scalar(b[:], v0[:], 0.0, 126.0, Alu.max, Alu.min)
                        d = scratch("d" + tag)
                        nc.vector.tensor_tensor(d[:], v0[:], b[:], Alu.subtract)
                        e0 = scratch("e0" + tag)
                        nc.vector.tensor_scalar(e0[:], d[:], 0.0, None, Alu.is_equal)
                        em = scratch("em" + tag)
                        nc.vector.tensor_scalar(em[:], d[:], -1.0, None, Alu.is_equal)
                        ep = scratch("ep" + tag)
                        nc.vector.tensor_scalar(ep[:], d[:], 1.0, None, Alu.is_equal)
                        ws0 = scratch("ws0" + tag)
                        t1 = scratch("t1" + tag)
                        nc.vector.tensor_tensor(ws0[:], W0[:], e0[:], Alu.mult)
                        nc.vector.tensor_tensor(t1[:], W1[:], em[:], Alu.mult)
                        nc.vector.tensor_tensor(ws0[:], ws0[:], t1[:], Alu.add)
                        ws1 = scratch("ws1" + tag)
                        t2 = scratch("t2" + tag)
                        nc.vector.tensor_tensor(ws1[:], W1[:], e0[:], Alu.mult)
                        nc.vector.tensor_tensor(t2[:], W0[:], ep[:], Alu.mult)
                        nc.vector.tensor_tensor(ws1[:], ws1[:], t2[:], Alu.add)
                        return b, ws0, ws1

                    by, wsy0, wsy1 = slot_weights(y0, A0, A1, "y")
                    bx, wsx0, wsx1 = slot_weights(x0, wx0, wx1, "x")

                    # quad weights wt[..., q=2*sy+sx] = wsy_sy * wsx_sx  (f16,
                    # q innermost for the Phase D combine)
                    for sy, Wy in ((0, wsy0), (1, wsy1)):
                        for sx, Wx in ((0, wsx0), (1, wsx1)):
                            nc.vector.tensor_tensor(
                                wt_t[:, :, g * GB:(g + 1) * GB, sy * 2 + sx],
                                Wy[:], Wx[:], Alu.mult)

                    # quad index = by*128 + bx
                    idxf = scratch("idxf")
                    nc.vector.tensor_scalar(idxf[:], by[:], 128.0, None, Alu.mult)
                    nc.vector.tensor_tensor(idxf[:], idxf[:], bx[:], Alu.add)
                    idx16 = wm.tile(shp, i16, tag="idx16", name="idx16")
                    nc.vector.tensor_copy(idx16[:], idxf[:])

                    # stage to DRAM [pp, k, blk-slice], reload [16p, k, h,
                    # blk] (blk innermost keeps runs contiguous), DVE-permute
                    # to gather order j = slot*128 + h*16 + p = slot*128 + pp,
                    # replicate x8 into the 128-partition wrap layout.
                    nc.sync.dma_start(
                        _ap(idx_d.ap(), g * GB,
                            [[K * BLK, 128], [BLK, K], [1, GB]]),
                        idx16[:])
                    idx_raw = wm.tile([16, K, 8, GB], i16, tag="iraw",
                                      name="idx_raw")
                    nc.sync.dma_start(
                        idx_raw[:],
                        _ap(idx_d.ap(), g * GB,
                            [[K * BLK, 16], [BLK, K], [16 * K * BLK, 8],
                             [1, GB]]))
                    idx_tr = idx_trs[g]
                    for k in range(K):
                        src = bass.AP(tensor=idx_raw[:].tensor,
                                      offset=idx_raw[:].offset + k * 8 * GB,
                                      ap=[list(idx_raw[:].ap[0]),
                                          [16, 2], [1, 16], [GB, 8]])
                        nc.vector.tensor_copy(idx_tr[:, k, :, :, :], src)
                    if g == 0:
                        for g8 in range(8):
                            nc.sync.dma_start(
                                idx_sb[g8 * 16:(g8 + 1) * 16, :, 0:256],
                                idx_tr[:].rearrange("p k c s h -> p k (c s h)"))

                if debug:
                    nc.sync.dma_start(dbg_conv.ap(), conv_sb[:])
                    nc.sync.dma_start(dbg_wt.ap(), wt_t[:])
                    nc.sync.dma_start(
                        _ap(dbg_idx.ap(), 0, [[1, 128 * K * BLK]]),
                        _ap(idx_d.ap(), 0, [[1, 128 * K * BLK]]))

            # ============ Phase D: gather + combine + GEMM ============
            with tc.tile_pool(name="gath", bufs=4) as gp, \
                 tc.tile_pool(name="vp", bufs=2) as vp, \
                 tc.tile_pool(name="vtp", bufs=2) as vtp, \
                 tc.tile_pool(name="oev", bufs=2) as op_, \
                 tc.tile_pool(name="pstr", bufs=1, space="PSUM") as pstr, \
                 tc.tile_pool(name="psout", bufs=1, space="PSUM") as pso:
                for ch in range(NCHUNK):
                    if ch == 1:
                        # group-1 idx replication, deferred so chunk 0's
                        # gathers only wait on group-0 staging
                        for g8 in range(8):
                            nc.sync.dma_start(
                                idx_sb[g8 * 16:(g8 + 1) * 16, :, 256:512],
                                idx_trs[1][:].rearrange(
                                    "p k c s h -> p k (c s h)"))
                    out_ps = pso.tile([O, CHUNK], f32)
                    for k in range(K):
                        gt = gp.tile([128, SLOTS, 512], f16, tag="g")
                        in_ap = _ap(xq_in.ap(), 0, [[512, HW], [1, 512]])
                        out_ap = _apf(gt[:], 0, [[512, SLOTS], [1, 512]])
                        nc.gpsimd.dma_gather(out_ap, in_ap,
                                             idx_sb[:, k, ch * 128:(ch + 1) * 128],
                                             num_idxs=CHUNK, num_idxs_reg=CHUNK,
                                             elem_size=512, elem_step=512,
                                             transpose=False,
                                             single_packet=False)
                        if debug and ch == 0 and k == 0:
                            nc.sync.dma_start(dbg_g.ap(), gt[:])
                        # gt element layout is [c, q] (q innermost): one
                        # 2x-mode multiply by the broadcast quad weights, then
                        # an innermost-axis add-reduce over q.
                        tmp = vp.tile([128, SLOTS, 128, 4], f16, tag="t")
                        val = vp.tile([128, SLOTS, 128], f32, tag="v")
                        wb = wt_t[:, k, ch * SLOTS:(ch + 1) * SLOTS, :][
                            :, :, None, :].to_broadcast((128, SLOTS, 128, 4))
                        gq = _apf(gt[:], 0, [[512, SLOTS], [4, 128], [1, 4]])
                        nc.vector.tensor_tensor(tmp[:], gq, wb, Alu.mult)
                        nc.vector.tensor_reduce(val[:], tmp[:],
                                                mybir.AxisListType.X, Alu.add)
                        if debug and ch == 0 and k == 0:
                            nc.sync.dma_start(dbg_val.ap(), val[:])
                        psT = pstr.tile([128, CHUNK], f32)
                        for j in range(SLOTS):
                            nc.tensor.transpose(psT[:, j * 128:(j + 1) * 128],
                                                val[:, j, :], identp[:])
                        valT = vtp.tile([128, CHUNK], f16, tag="vt")
                        nc.scalar.activation(valT[:], psT[:], Act.Copy)
                        for b in range(CHUNK // 512):
                            nc.tensor.matmul(
                                out_ps[:, b * 512:(b + 1) * 512],
                                w2_t[:, k, :],
                                valT[:, b * 512:(b + 1) * 512],
                                start=(k == 0), stop=(k == K - 1))
                    ot = op_.tile([O, CHUNK], f16, tag="o")
                    nc.scalar.activation(ot[:], out_ps[:], Act.Copy)
                    nc.sync.dma_start(
                        _ap(out_o.ap(), ch * CHUNK, [[NPX, O], [1, CHUNK]]),
                        ot[:])
    nc.compile()
    return nc


def _host_inputs(x, w_off, b_off, w_mod, b_mod, w_reg):
    """Build the 8 per-core input maps."""
    # conv weights reordered: [off_y(9), off_x(9), mask(9)]
    wcat = np.concatenate([w_off[0::2], w_off[1::2], w_mod], axis=0)  # [27,128,3,3]
    bcat = np.concatenate([b_off[0::2], b_off[1::2], b_mod], axis=0)  # [27]
    wconv = np.ascontiguousarray(
        wcat.transpose(1, 2, 3, 0).reshape(C, K * NCH)).astype(np.float16)
    bias = bcat.reshape(NCH, 1).astype(np.float32)
    w2 = np.ascontiguousarray(
        (w_reg * 2.0).transpose(1, 2, 3, 0).reshape(C, K * O)).astype(np.float16)
    ki = np.arange(K) // 3
    kj = np.arange(K) % 3
    basex = (np.arange(128)[:, None] + kj[None, :] - 1).astype(np.float32)

    # corner-quad layout per batch, q innermost: xq[y*128+x][c][q] with
    # q = [x(y,x), x(y,x+1), x(y+1,x), x(y+1,x+1)][c]
    B = x.shape[0]
    xf = x.astype(np.float16)
    xq_all = []
    for b in range(B):
        xp = np.zeros((129, 129, C), dtype=np.float16)
        xp[:128, :128] = xf[b].transpose(1, 2, 0)
        quad = np.stack([xp[:128, :128], xp[:128, 1:129],
                         xp[1:129, :128], xp[1:129, 1:129]], axis=-1)
        xq_all.append(np.ascontiguousarray(quad.reshape(HW * 4 * C)))

    maps = []
    for core in range(8):
        b, hf = core // 2, core % 2
        xpadfull = np.zeros((C, 130, 130), dtype=np.float16)
        xpadfull[:, 1:129, 1:129] = xf[b]
        xpad = np.ascontiguousarray(xpadfull[:, 64 * hf:64 * hf + 66, :])
        rloc = 64 * hf + np.arange(BLK)
        basey = np.broadcast_to(
            (rloc[None, :] + ki[:, None] - 1)[None, :, :],
            (128, K, BLK)).reshape(128, K * BLK).astype(np.float32)
        maps.append({
            "xq": xq_all[b],
            "xpad": xpad.reshape(C, 66 * 130),
            "wconv": wconv,
            "bias": bias,
            "w2": w2,
            "basey": np.ascontiguousarray(basey),
            "basex": basex,
        })
    return maps


_NC_CACHE = {}


def kernel(x, w_off, b_off, w_mod, b_mod, w_reg, debug=False, trace=False):
    x = np.asarray(x)
    key = ("nc", debug)
    if key not in _NC_CACHE:
        _NC_CACHE[key] = build_kernel(debug=debug)
    nc = _NC_CACHE[key]
    maps = _host_inputs(x, np.asarray(w_off), np.asarray(b_off),
                        np.asarray(w_mod), np.asarray(b_mod), np.asarray(w_reg))
    res = run_bass_kernel_spmd(nc, maps, core_ids=list(range(8)), trace=trace)
    B = x.shape[0]
    out = np.empty((B, O, H, W), dtype=np.float32)
    for core in range(8):
        b, hf = core // 2, core % 2
        out[b, :, 64 * hf:64 * (hf + 1), :] = \
            res.results[core]["out"].astype(np.float32).reshape(O, BLK, 128)
    kernel._last_results = res
    return out
